# revision 14
# baseline (speedup 1.0000x reference)
"""BlurPool3D Trainium2 kernel.

Depthwise 3x3x3 separable (rank-1) blur, stride 2, pad 1 on
x[2, 64, 64, 96, 96] f32 -> y[2, 64, 32, 48, 48] f32.

Strategy (8 NeuronCores, SPMD, DMA-roofline oriented):
  - Shard the 128 (n, c) pairs across cores: 16 per core, 8 blocks of
    2 channels. Channels are independent in a depthwise conv -> no
    halo, no collectives.
  - Per block, SBUF partitions = (2 nc x 64 d) = 128, free = spatial.
    The full D axis lives on partitions, so the D-tap contraction is a
    matmul with a block-diagonal band lhsT; the 3 H taps are 3
    accumulating matmuls reading h-shifted rows. D/H edges are handled
    by the band matrix / a zeroed pad row.
  - W-pass on VectorE in exactly 3 ops per 49-row half-tile (one
    tensor_scalar for the w'=0 edge column, two fused
    scalar_tensor_tensor for the rest), all fp32 so the DVE two-port
    fast path stays engaged.
  - Matmuls run in bf16 (exact for the binomial taps), accumulating
    in fp32 PSUM over 3 H taps per 8-row chunk. The two h-halves map to PE column groups
    0/1 (tile_position) writing PSUM partitions 0-63/64-127; ScalarE
    drains PSUM -> SBUF fp32.
  - All input DMAs ride the SP ring (nc.sync) so prefetch is never
    head-of-line blocked behind an output DMA waiting on compute;
    drains + output DMAs ride the ACT ring (nc.scalar). One
    128-partition output DMA per block.
"""

import os
import sys

for _p in ("/opt/trn_rl_repo",):
    if _p not in sys.path and os.path.isdir(_p):
        sys.path.insert(0, _p)

import numpy as np

N, C, D, H, W = 2, 64, 64, 96, 96
DO, HO, WO = 32, 48, 48
NCORES = 8
NC_PER_CORE = (N * C) // NCORES  # 16
BLOCKS = NC_PER_CORE // 2  # 8 blocks of 2 channels each

_PROGRAM_CACHE = {}


def _rank1_factors(filt):
    """Per-channel rank-1 factorization filt[c,0] = outer(d, h, w)."""
    dvec = np.empty((C, 3), np.float64)
    hvec = np.empty((C, 3), np.float64)
    wvec = np.empty((C, 3), np.float64)
    for c in range(C):
        T = filt[c, 0].astype(np.float64)
        idx = np.unravel_index(np.argmax(np.abs(T)), T.shape)
        i0, j0, k0 = idx
        piv = T[i0, j0, k0]
        if piv == 0.0:
            dvec[c] = hvec[c] = wvec[c] = 0.0
            continue
        dvec[c] = T[:, j0, k0]
        hvec[c] = T[i0, :, k0] / piv
        wvec[c] = T[i0, j0, :] / piv
        recon = np.einsum("i,j,k->ijk", dvec[c], hvec[c], wvec[c])
        resid = np.abs(recon - T).max()
        if resid > 1e-6 * max(np.abs(T).max(), 1e-30):
            raise ValueError(f"filter channel {c} is not rank-1 (resid {resid})")
    return dvec, hvec, wvec


def _build_program(uniform):
    import concourse.bacc as bacc
    import concourse.mybir as mybir
    from concourse import tile

    dt = mybir.dt
    nc = bacc.Bacc("TRN2", target_bir_lowering=False, debug=False,
                   num_devices=NCORES)

    nbm = 1 if uniform else BLOCKS
    x = nc.dram_tensor("x", [NC_PER_CORE, D, H * W], dt.float32,
                       kind="ExternalInput")
    bmat = nc.dram_tensor("bmat", [128, nbm * 3 * 64], dt.bfloat16,
                          kind="ExternalInput")
    wtaps = nc.dram_tensor("wtaps", [128, 2 * BLOCKS], dt.float32,
                           kind="ExternalInput")
    y = nc.dram_tensor("y", [NC_PER_CORE, DO, HO * WO], dt.float32,
                       kind="ExternalOutput")

    # 48 output h-rows per block: two halves (g) of 24, three 8-row PSUM
    # chunks each; every chunk is 3 accumulating H-tap matmuls.
    CH = 8
    CHUNKS = [(0, CH), (8, CH), (16, CH)]

    with tile.TileContext(nc) as tc:
        with tc.tile_pool(name="const", bufs=1) as cpool, \
             tc.tile_pool(name="xp", bufs=6) as xpool, \
             tc.tile_pool(name="pf", bufs=3) as pfpool, \
             tc.tile_pool(name="pp", bufs=4) as ppool, \
             tc.tile_pool(name="op", bufs=3) as opool, \
             tc.tile_pool(name="ps", bufs=6, space="PSUM") as pspool:
            bt = cpool.tile([128, nbm * 3 * 64], dt.bfloat16)
            wt = cpool.tile([128, 2 * BLOCKS], dt.float32)
            nc.scalar.dma_start(bt[:], bmat[:])
            nc.scalar.dma_start(wt[:], wtaps[:])

            for b in range(BLOCKS):
                bcol = 0 if uniform else b * 3 * 64
                r1 = wt[:, 2 * b:2 * b + 1]
                r2 = wt[:, 2 * b + 1:2 * b + 2]
                src = x[2 * b:2 * b + 2].rearrange("a d f -> (a d) f")
                src = src.rearrange("p (h w) -> p h w", h=H)

                # x tiles: one 49-row tile per h-half g; row j maps to
                # x row 48g - 1 + j. g=0 row 0 is the zero H-pad row.
                xts = []
                for g in range(2):
                    xt = xpool.tile([128, 49, W], dt.float32, tag="xt")
                    if g == 0:
                        nc.gpsimd.memset(xt[:, 0:1, :], 0.0)
                        if b == 0:
                            # Split the very first loads so the W-pass can
                            # start on rows 0-27 while rows 28+ stream in
                            # (the HWDGE ring interleaves queued DMAs, so
                            # one big DMA completes late).
                            nc.sync.dma_start(xt[:, 1:28, :],
                                              src[:, 0:27, :])
                            nc.sync.dma_start(xt[:, 28:49, :],
                                              src[:, 27:48, :])
                        else:
                            nc.sync.dma_start(xt[:, 1:49, :],
                                              src[:, 0:48, :])
                    else:
                        if b == 0:
                            nc.sync.dma_start(xt[:, 0:28, :],
                                              src[:, 47:75, :])
                            nc.sync.dma_start(xt[:, 28:49, :],
                                              src[:, 75:96, :])
                        else:
                            nc.sync.dma_start(xt[:], src[:, 47:96, :])
                    xts.append(xt)

                pss = [pspool.tile([128, CH * WO], dt.float32, tag="ps",
                                   name="ps")
                       for _ in CHUNKS]
                for g in range(2):
                    # W-pass: p[j, w'] = x[j, 2w'-1] + r1*x[j, 2w'] +
                    # r2*x[j, 2w'+1] (w'=0 left tap is zero-pad), bf16
                    # out for the PE. Each op is split at row 28: the
                    # DVE fast path only engages below ~2048 free
                    # elements per instruction (1316/987 < 2048 < 2303).
                    # Two-step W-pass, split at row 28 (the DVE fast
                    # path needs small ops): step 1 keeps the partial sum
                    # in fp32 (all-fp32 DVE ops run ~2x faster than ones
                    # with any bf16 operand), step 2 adds the last tap and
                    # converts to bf16 for the PE on the way out.
                    pf = pfpool.tile([128, 49, WO], dt.float32, tag="pf")
                    p = ppool.tile([128, 49, WO], dt.bfloat16, tag="p")
                    nc.vector.tensor_scalar(
                        pf[:, :, 0:1], xts[g][:, :, 0:1], r1, None,
                        mybir.AluOpType.mult)
                    for ra, rb in ((0, 28), (28, 49)):
                        nc.vector.scalar_tensor_tensor(
                            pf[:, ra:rb, 1:WO],
                            xts[g][:, ra:rb, 2:2 * WO - 1:2], r1,
                            xts[g][:, ra:rb, 1:2 * WO - 2:2],
                            mybir.AluOpType.mult, mybir.AluOpType.add)
                        nc.vector.scalar_tensor_tensor(
                            p[:, ra:rb, 0:WO],
                            xts[g][:, ra:rb, 1:2 * WO:2], r2,
                            pf[:, ra:rb, 0:WO],
                            mybir.AluOpType.mult, mybir.AluOpType.add)
                    # Fused H+D matmuls: the two h-halves map to PE
                    # column groups 0/1 writing PSUM partitions 0-63 /
                    # 64-127; chunk-major so each chunk closes early.
                    for ci, (h0, cnt) in enumerate(CHUNKS):
                        psv = pss[ci]
                        for k in range(3):
                            lhsT = bt[:, bcol + k * 64:bcol + (k + 1) * 64]
                            rhs = p[:, 2 * h0 + k:2 * h0 + k + 2 * cnt - 1:2, :]
                            nc.tensor.matmul(
                                psv[64:, :] if g else psv[:64, :],
                                lhsT, rhs,
                                start=(k == 0), stop=(k == 2),
                                tile_position=(0, 64 * g) if g else None)

                # PSUM -> SBUF drain on ScalarE, then per-half output
                # DMAs: partition (g, ncl, d') lands on
                # y[2b+ncl, d', g-half rows].
                ot = opool.tile([128, HO * WO // 2], dt.float32)
                for ci, (h0, cnt) in enumerate(CHUNKS):
                    nc.scalar.copy(ot[:, h0 * WO:(h0 + cnt) * WO],
                                   pss[ci][:])
                for g in range(2):
                    dst = y[2 * b:2 * b + 2, :, g * HO * WO // 2:
                            (g + 1) * HO * WO // 2]
                    dst = dst.rearrange("a d f -> (a d) f")
                    nc.scalar.dma_start(dst, ot[g * 64:(g + 1) * 64, :])
    nc.compile()
    return nc


def kernel(x, filt):
    x = np.ascontiguousarray(np.asarray(x, dtype=np.float32))
    filt = np.asarray(filt, dtype=np.float32)
    assert x.shape == (N, C, D, H, W), x.shape

    import ml_dtypes
    from concourse.bass_utils import run_bass_kernel_spmd

    dvec, hvec, wvec = _rank1_factors(filt)
    # W pivot (left tap w0) folded into the matmul matrices.
    w0 = wvec[:, 0].copy()
    safe = np.abs(w0) > 1e-30
    if not safe.all():
        raise ValueError("W-tap pivot is zero; unsupported filter")
    r1 = wvec[:, 1] / w0
    r2 = wvec[:, 2] / w0

    uniform = bool(np.all(filt == filt[:1]))
    xr = x.reshape(N * C, D, H * W)

    in_maps = []
    for core in range(NCORES):
        chans = (np.arange(NC_PER_CORE) + core * NC_PER_CORE) % C  # local->c
        wt = np.empty((128, 2 * BLOCKS), np.float32)
        bm = np.zeros((128, (1 if uniform else BLOCKS) * 3 * 64), np.float64)
        for b in range(BLOCKS):
            for ncl in range(2):
                c = chans[2 * b + ncl]
                wt[ncl * 64:(ncl + 1) * 64, 2 * b + 0] = r1[c]
                wt[ncl * 64:(ncl + 1) * 64, 2 * b + 1] = r2[c]
                if uniform and b > 0:
                    continue
                # band matrix rows (ncl*64 + d), cols (ncl*32 + d'),
                # one 64-col group per H tap k, scaled by hvec[k] and
                # the W pivot w0.
                for k in range(3):
                    col0 = (b * 3 + k) * 64 + ncl * 32
                    for dp in range(DO):
                        for delta in range(3):
                            d = 2 * dp - 1 + delta
                            if 0 <= d < D:
                                bm[ncl * 64 + d, col0 + dp] = (
                                    dvec[c, delta] * hvec[c, k] * w0[c])
        in_maps.append({
            "x": np.ascontiguousarray(
                xr[core * NC_PER_CORE:(core + 1) * NC_PER_CORE]),
            "bmat": bm.astype(ml_dtypes.bfloat16),
            "wtaps": wt,
        })

    key = ("prog", uniform)
    if key not in _PROGRAM_CACHE:
        _PROGRAM_CACHE[key] = _build_program(uniform)
    nc = _PROGRAM_CACHE[key]

    trace = bool(int(os.environ.get("BLURPOOL_TRACE", "0")))
    kwargs = {}
    if trace and os.environ.get("BLURPOOL_TRACE_DIR"):
        kwargs["tmpdir"] = os.environ["BLURPOOL_TRACE_DIR"]
    res = run_bass_kernel_spmd(nc, in_maps, core_ids=list(range(NCORES)),
                               trace=trace, **kwargs)
    if trace:
        kernel.last_result = res

    out = np.concatenate([r["y"].reshape(NC_PER_CORE, DO, HO, WO)
                          for r in res.results], axis=0)
    return np.ascontiguousarray(out.reshape(N, C, DO, HO, WO))


# revision 15
# speedup vs baseline: 1.1169x; 1.1169x over previous
"""BlurPool3D Trainium2 kernel.

Depthwise 3x3x3 separable (rank-1) blur, stride 2, pad 1 on
x[2, 64, 64, 96, 96] f32 -> y[2, 64, 32, 48, 48] f32.

Strategy (8 NeuronCores, SPMD, DMA-roofline oriented):
  - Shard the 128 (n, c) pairs across cores: 16 per core, 8 blocks of
    2 channels. Channels are independent in a depthwise conv -> no
    halo, no collectives.
  - Per block, SBUF partitions = (2 nc x 64 d) = 128, free = spatial.
    The full D axis lives on partitions, so the D-tap contraction is a
    matmul with a block-diagonal band lhsT; the 3 H taps are 3
    accumulating matmuls reading h-shifted rows. D/H edges are handled
    by the band matrix / a zeroed pad row.
  - W-pass on VectorE in exactly 3 ops per 49-row half-tile (one
    tensor_scalar for the w'=0 edge column, two fused
    scalar_tensor_tensor for the rest), all fp32 so the DVE two-port
    fast path stays engaged.
  - Matmuls run in bf16 (exact for the binomial taps), accumulating
    in fp32 PSUM over 3 H taps per 8-row chunk. The two h-halves map to PE column groups
    0/1 (tile_position) writing PSUM partitions 0-63/64-127; ScalarE
    drains PSUM -> SBUF fp32.
  - All input DMAs ride the SP ring (nc.sync) so prefetch is never
    head-of-line blocked behind an output DMA waiting on compute;
    drains + output DMAs ride the ACT ring (nc.scalar). One
    128-partition output DMA per block.
"""

import os
import sys

for _p in ("/opt/trn_rl_repo",):
    if _p not in sys.path and os.path.isdir(_p):
        sys.path.insert(0, _p)

import numpy as np

N, C, D, H, W = 2, 64, 64, 96, 96
DO, HO, WO = 32, 48, 48
NCORES = 8
NC_PER_CORE = (N * C) // NCORES  # 16
BLOCKS = NC_PER_CORE // 2  # 8 blocks of 2 channels each

_PROGRAM_CACHE = {}


def _rank1_factors(filt):
    """Per-channel rank-1 factorization filt[c,0] = outer(d, h, w)."""
    dvec = np.empty((C, 3), np.float64)
    hvec = np.empty((C, 3), np.float64)
    wvec = np.empty((C, 3), np.float64)
    for c in range(C):
        T = filt[c, 0].astype(np.float64)
        idx = np.unravel_index(np.argmax(np.abs(T)), T.shape)
        i0, j0, k0 = idx
        piv = T[i0, j0, k0]
        if piv == 0.0:
            dvec[c] = hvec[c] = wvec[c] = 0.0
            continue
        dvec[c] = T[:, j0, k0]
        hvec[c] = T[i0, :, k0] / piv
        wvec[c] = T[i0, j0, :] / piv
        recon = np.einsum("i,j,k->ijk", dvec[c], hvec[c], wvec[c])
        resid = np.abs(recon - T).max()
        if resid > 1e-6 * max(np.abs(T).max(), 1e-30):
            raise ValueError(f"filter channel {c} is not rank-1 (resid {resid})")
    return dvec, hvec, wvec


def _build_program(uniform):
    import concourse.bacc as bacc
    import concourse.mybir as mybir
    from concourse import tile

    dt = mybir.dt
    nc = bacc.Bacc("TRN2", target_bir_lowering=False, debug=False,
                   num_devices=NCORES)

    nbm = 1 if uniform else BLOCKS
    x = nc.dram_tensor("x", [NC_PER_CORE, D, H * W], dt.float32,
                       kind="ExternalInput")
    bmat = nc.dram_tensor("bmat", [128, nbm * 3 * 64], dt.bfloat16,
                          kind="ExternalInput")
    wtaps = nc.dram_tensor("wtaps", [128, 2 * BLOCKS], dt.float32,
                           kind="ExternalInput")
    y = nc.dram_tensor("y", [NC_PER_CORE, DO, HO * WO], dt.float32,
                       kind="ExternalOutput")

    # 48 output h-rows per block: two halves (g) of 24, three 8-row PSUM
    # chunks each; every chunk is 3 accumulating H-tap matmuls.
    CH = 8
    CHUNKS = [(0, CH), (8, CH), (16, CH)]

    with tile.TileContext(nc) as tc:
        with tc.tile_pool(name="const", bufs=1) as cpool, \
             tc.tile_pool(name="xp", bufs=6) as xpool, \
             tc.tile_pool(name="pp", bufs=4) as ppool, \
             tc.tile_pool(name="op", bufs=3) as opool, \
             tc.tile_pool(name="ps", bufs=6, space="PSUM") as pspool:
            bt = cpool.tile([128, nbm * 3 * 64], dt.bfloat16)
            wt = cpool.tile([128, 2 * BLOCKS], dt.float32)
            nc.scalar.dma_start(bt[:], bmat[:])
            nc.scalar.dma_start(wt[:], wtaps[:])

            for b in range(BLOCKS):
                bcol = 0 if uniform else b * 3 * 64
                r1 = wt[:, 2 * b:2 * b + 1]
                r2 = wt[:, 2 * b + 1:2 * b + 2]
                src = x[2 * b:2 * b + 2].rearrange("a d f -> (a d) f")
                src = src.rearrange("p (h w) -> p h w", h=H)

                # x tiles: one 49-row tile per h-half g; row j maps to
                # x row 48g - 1 + j. g=0 row 0 is the zero H-pad row.
                # b0 and b7 stream their tiles in three row chunks:
                # b0 so the W-pass starts while the rest arrives (queued
                # HWDGE DMAs interleave, delaying single big transfers),
                # b7 so the pipeline tail overlaps the last arrivals.
                split = b in (0, BLOCKS - 1)
                xts = []
                for g in range(2):
                    xt = xpool.tile([128, 49, W], dt.float32, tag="xt")
                    lo = 1 if g == 0 else 0
                    off = -1 if g == 0 else 47
                    if g == 0:
                        nc.gpsimd.memset(xt[:, 0:1, :], 0.0)
                    if split:
                        for ra, rb in ((lo, 18), (18, 34), (34, 49)):
                            nc.sync.dma_start(xt[:, ra:rb, :],
                                              src[:, off + ra:off + rb, :])
                    else:
                        nc.sync.dma_start(xt[:, lo:49, :],
                                          src[:, off + lo:off + 49, :])
                    xts.append(xt)

                pss = [pspool.tile([128, CH * WO], dt.float32, tag="ps",
                                   name="ps")
                       for _ in CHUNKS]
                for g in range(2):
                    # W-pass: p[j, w'] = x[j, 2w'-1] + r1*x[j, 2w'] +
                    # r2*x[j, 2w'+1] (w'=0 left tap is zero-pad), bf16
                    # out for the PE. Each op is split at row 28: the
                    # DVE fast path only engages below ~2048 free
                    # elements per instruction (1316/987 < 2048 < 2303).
                    # W-pass: p[j, w'] = x[j, 2w'-1] + r1*x[j, 2w'] +
                    # r2*x[j, 2w'+1] (w'=0 left tap is zero-pad), bf16
                    # out for the PE. Unsplit ops minimize instruction
                    # count and SBUF activity (the HW activity monitor
                    # throttles under load); b0/b7 split to track their
                    # chunked DMAs.
                    p = ppool.tile([128, 49, WO], dt.bfloat16, tag="p")
                    nc.vector.tensor_scalar(
                        p[:, :, 0:1], xts[g][:, :, 0:1], r1, None,
                        mybir.AluOpType.mult)
                    ranges = ((0, 18), (18, 34), (34, 49)) if split \
                        else ((0, 49),)
                    for ra, rb in ranges:
                        nc.vector.scalar_tensor_tensor(
                            p[:, ra:rb, 1:WO],
                            xts[g][:, ra:rb, 2:2 * WO - 1:2], r1,
                            xts[g][:, ra:rb, 1:2 * WO - 2:2],
                            mybir.AluOpType.mult, mybir.AluOpType.add)
                        nc.vector.scalar_tensor_tensor(
                            p[:, ra:rb, 0:WO],
                            xts[g][:, ra:rb, 1:2 * WO:2], r2,
                            p[:, ra:rb, 0:WO],
                            mybir.AluOpType.mult, mybir.AluOpType.add)
                    # Fused H+D matmuls: the two h-halves map to PE
                    # column groups 0/1 writing PSUM partitions 0-63 /
                    # 64-127; chunk-major so each chunk closes early.
                    for ci, (h0, cnt) in enumerate(CHUNKS):
                        psv = pss[ci]
                        for k in range(3):
                            lhsT = bt[:, bcol + k * 64:bcol + (k + 1) * 64]
                            rhs = p[:, 2 * h0 + k:2 * h0 + k + 2 * cnt - 1:2, :]
                            nc.tensor.matmul(
                                psv[64:, :] if g else psv[:64, :],
                                lhsT, rhs,
                                start=(k == 0), stop=(k == 2),
                                tile_position=(0, 64 * g) if g else None)

                # PSUM -> SBUF drain on ScalarE, then per-half output
                # DMAs: partition (g, ncl, d') lands on
                # y[2b+ncl, d', g-half rows].
                ot = opool.tile([128, HO * WO // 2], dt.float32)
                for ci, (h0, cnt) in enumerate(CHUNKS):
                    nc.scalar.copy(ot[:, h0 * WO:(h0 + cnt) * WO],
                                   pss[ci][:])
                    if b == BLOCKS - 1:
                        # last block: ship each chunk as soon as its
                        # drain lands instead of waiting for the full tile
                        for g in range(2):
                            dst = y[2 * b:2 * b + 2, :,
                                    g * HO * WO // 2 + h0 * WO:
                                    g * HO * WO // 2 + (h0 + cnt) * WO]
                            dst = dst.rearrange("a d f -> (a d) f")
                            nc.scalar.dma_start(
                                dst, ot[g * 64:(g + 1) * 64,
                                        h0 * WO:(h0 + cnt) * WO])
                if b < BLOCKS - 1:
                    for g in range(2):
                        dst = y[2 * b:2 * b + 2, :, g * HO * WO // 2:
                                (g + 1) * HO * WO // 2]
                        dst = dst.rearrange("a d f -> (a d) f")
                        nc.scalar.dma_start(dst, ot[g * 64:(g + 1) * 64, :])
    nc.compile()
    return nc


def kernel(x, filt):
    x = np.ascontiguousarray(np.asarray(x, dtype=np.float32))
    filt = np.asarray(filt, dtype=np.float32)
    assert x.shape == (N, C, D, H, W), x.shape

    import ml_dtypes
    from concourse.bass_utils import run_bass_kernel_spmd

    dvec, hvec, wvec = _rank1_factors(filt)
    # W pivot (left tap w0) folded into the matmul matrices.
    w0 = wvec[:, 0].copy()
    safe = np.abs(w0) > 1e-30
    if not safe.all():
        raise ValueError("W-tap pivot is zero; unsupported filter")
    r1 = wvec[:, 1] / w0
    r2 = wvec[:, 2] / w0

    uniform = bool(np.all(filt == filt[:1]))
    xr = x.reshape(N * C, D, H * W)

    in_maps = []
    for core in range(NCORES):
        chans = (np.arange(NC_PER_CORE) + core * NC_PER_CORE) % C  # local->c
        wt = np.empty((128, 2 * BLOCKS), np.float32)
        bm = np.zeros((128, (1 if uniform else BLOCKS) * 3 * 64), np.float64)
        for b in range(BLOCKS):
            for ncl in range(2):
                c = chans[2 * b + ncl]
                wt[ncl * 64:(ncl + 1) * 64, 2 * b + 0] = r1[c]
                wt[ncl * 64:(ncl + 1) * 64, 2 * b + 1] = r2[c]
                if uniform and b > 0:
                    continue
                # band matrix rows (ncl*64 + d), cols (ncl*32 + d'),
                # one 64-col group per H tap k, scaled by hvec[k] and
                # the W pivot w0.
                for k in range(3):
                    col0 = (b * 3 + k) * 64 + ncl * 32
                    for dp in range(DO):
                        for delta in range(3):
                            d = 2 * dp - 1 + delta
                            if 0 <= d < D:
                                bm[ncl * 64 + d, col0 + dp] = (
                                    dvec[c, delta] * hvec[c, k] * w0[c])
        in_maps.append({
            "x": np.ascontiguousarray(
                xr[core * NC_PER_CORE:(core + 1) * NC_PER_CORE]),
            "bmat": bm.astype(ml_dtypes.bfloat16),
            "wtaps": wt,
        })

    key = ("prog", uniform)
    if key not in _PROGRAM_CACHE:
        _PROGRAM_CACHE[key] = _build_program(uniform)
    nc = _PROGRAM_CACHE[key]

    trace = bool(int(os.environ.get("BLURPOOL_TRACE", "0")))
    kwargs = {}
    if trace and os.environ.get("BLURPOOL_TRACE_DIR"):
        kwargs["tmpdir"] = os.environ["BLURPOOL_TRACE_DIR"]
    res = run_bass_kernel_spmd(nc, in_maps, core_ids=list(range(NCORES)),
                               trace=trace, **kwargs)
    if trace:
        kernel.last_result = res

    out = np.concatenate([r["y"].reshape(NC_PER_CORE, DO, HO, WO)
                          for r in res.results], axis=0)
    return np.ascontiguousarray(out.reshape(N, C, DO, HO, WO))
